# revision 20
# baseline (speedup 1.0000x reference)
"""MoE ragged FFN kernel for Trainium2 (8 NeuronCores, expert-parallel).

Strategy
--------
* Router (RMSNorm -> scaled projection -> softmax -> top-k -> renorm) is
  computed on host with jax-on-CPU using exactly the reference ops, so the
  discrete expert choices match the reference bit-for-bit.
* Expert-parallel sharding: core e owns expert e's weights. Tokens routed to
  expert e are gathered on host, padded to a common capacity C, and shipped
  pre-transposed so the device only runs dense matmuls.
* Device (per core): Y1^T = Wg^T @ X^T (contraction F), GLU
  act = gelu_tanh(gate) * lin computed pairwise on 128-row tiles,
  Y2^T = Wl^T @ act (contraction H). Tokens are always the matmul moving/free
  dimension; features live on partitions. fp16 matmul inputs (value ranges
  here are far from fp16 limits; 8x finer quantization than bf16 at the same
  1 cycle/row PE rate), fp32 PSUM accumulate.
* Default structure ("dram_act") stages activations through DRAM: stage-1
  streams wg exactly once with x SBUF-resident; stage-2 splits H into
  quarters with the act quarter SBUF-resident, wl streamed once, and four
  fp16 partial outputs summed in fp32 on host.
* Trace-driven scheduling: batched x loads (HWDGE issue is ~0.6us per
  dma_start regardless of size), phase-2 act/wl prefetch during phase 1
  (pools hoisted to top level to avoid pool-release dependencies),
  per-phase PSUM pool scoping (7 banks stage 1 / 8 banks stage 2), PE
  warm-up matmuls sized to bridge until the first x chunk lands.
* Host combines: out[token] += combine_weight * per_expert_scale[e] * y.

Measured (8 cores, G=4 S=2048 F=2048 H=4096 E=8 k=2): HW exec ~1.42 ms
(PE idle <5us, ~96.4% MFU, ~104% of the C=2142 single-core matmul-stream
floor), global L2 relative error ~5.5e-4 vs the fp32 reference.
"""

import os

import numpy as np

P = 128
RMS_EPS = 1e-6

# Matmul input precision: "f16" (default), "bf16", or "f32r".
MOE_DTYPE = os.environ.get("MOE_DTYPE", "f16")
# Token-block size (matmul moving free dim; >=256 keeps fp32r at 1 cyc/row).
TB = int(os.environ.get("MOE_TB", "512"))

_NEFF_CACHE: dict = {}


def _route_numpy(x, w_router, router_scale, top_k):
    """Fallback router in numpy (used only if jax-on-CPU is unavailable)."""
    G, S, F = x.shape
    B = G * S
    var = np.mean(np.square(x), axis=-1, keepdims=True, dtype=np.float32)
    ri = x / np.sqrt(var + RMS_EPS)
    ri = ri * np.float32(1.0 / np.sqrt(np.float32(F))) * router_scale
    logits = (ri.reshape(B, F) @ w_router).astype(np.float32)
    m = logits.max(axis=-1, keepdims=True)
    e = np.exp(logits - m)
    probs = e / e.sum(axis=-1, keepdims=True)
    choices = np.argsort(-logits, axis=-1, kind="stable")[:, :top_k]
    sel = np.take_along_axis(probs, choices, axis=-1)
    renorm = sel.sum(axis=-1, keepdims=True)
    renorm = np.where(renorm > 0.0, renorm, np.float32(1.0))
    combine = (sel / renorm).astype(np.float32)
    return choices.astype(np.int64), combine


def _route(x, w_router, router_scale, top_k):
    """Reference-exact router on CPU via jax. Returns (choices, combine) as
    numpy arrays of shape (B, k)."""
    try:
        import jax
        import jax.numpy as jnp

        cpu = jax.devices("cpu")[0]
    except Exception:
        return _route_numpy(np.asarray(x, dtype=np.float32),
                            np.asarray(w_router), np.asarray(router_scale),
                            top_k)
    G, S, F = x.shape
    E = w_router.shape[1]
    with jax.default_device(cpu):
        xj = jax.device_put(np.asarray(x), cpu)
        wj = jax.device_put(np.asarray(w_router), cpu)
        rj = jax.device_put(np.asarray(router_scale), cpu)
        var = jnp.mean(jnp.square(xj), axis=-1, keepdims=True)
        ri = xj * jax.lax.rsqrt(var + RMS_EPS)
        root_size = jax.lax.rsqrt(jnp.array(F, dtype=ri.dtype))
        ri = ri * root_size * rj.astype(ri.dtype)
        logits = jnp.einsum("gsd,de->gse", ri, wj).astype(jnp.float32)
        probs = jax.nn.softmax(logits, axis=-1)
        _, choices = jax.lax.approx_max_k(logits, k=top_k)
        indicator = jax.nn.one_hot(choices, E, dtype=probs.dtype).sum(axis=-2)
        renorm = jnp.sum(indicator * probs, axis=-1, keepdims=True)
        renorm = jnp.where(renorm > 0.0, renorm, 1.0)
        weights = probs / renorm
        combine = jnp.take_along_axis(weights, choices, axis=-1)
    B = G * S
    return (
        np.asarray(choices).reshape(B, top_k),
        np.asarray(combine).reshape(B, top_k).astype(np.float32),
    )


def _build_nc(C, F, H, dtype_name):
    """Build + compile the per-core FFN program (same program on all cores)."""
    import concourse.mybir as mybir
    import concourse.tile as tile
    from concourse import bacc

    KF = F // P          # k-subtiles for stage 1 (contraction F)
    KH = H // P          # k-subtiles for stage 2 (contraction H)
    MG = 2 * H // P      # wg column tiles, gate/lin interleaved per 128
    MO = F // P          # output row tiles
    f32 = mybir.dt.float32
    dt_in = _mm_dt(mybir, dtype_name)
    dt_mm = dt_in

    def mm(ap):
        return ap.bitcast(dt_mm) if ap.dtype != dt_mm else ap

    nc = bacc.Bacc(None, target_bir_lowering=False)
    xT = nc.dram_tensor("xT", [P, KF, C], dt_in, kind="ExternalInput")
    wg = nc.dram_tensor("wg", [P, MG, KF, P], dt_in, kind="ExternalInput")
    wl = nc.dram_tensor("wl", [P, MO, KH, P], dt_in, kind="ExternalInput")
    yT = nc.dram_tensor("yT", [MO, P, C], f32, kind="ExternalOutput")

    # Equal-size blocks (<= TB): avoids a tiny LDWEIGHTS-bound tail block.
    # Sizes are kept even (fp32r matmuls require an even moving free dim).
    assert C % 2 == 0
    nblk = -(-C // TB)
    half_base, half_extra = divmod(C // 2, nblk)
    blocks = []
    c0 = 0
    for b in range(nblk):
        tb = 2 * (half_base + (1 if b < half_extra else 0))
        blocks.append((c0, tb))
        c0 += tb
    assert c0 == C

    # f32r tiles are 2x the size of bf16 — shrink pools to fit SBUF.
    wbufs = 4 if dtype_name != "f32r" else 2
    abufs = 2 if dtype_name != "f32r" else 1
    with tile.TileContext(nc) as tc:
        with (
            tc.tile_pool(name="xp", bufs=2) as xp,
            tc.tile_pool(name="wgp", bufs=wbufs) as wgp,
            tc.tile_pool(name="wlp", bufs=wbufs) as wlp,
            tc.tile_pool(name="actp", bufs=abufs) as actp,
            tc.tile_pool(name="gp", bufs=3) as gp,
            tc.tile_pool(name="op", bufs=3) as op,
            tc.tile_pool(name="ps1", bufs=4, space="PSUM") as ps1,
            tc.tile_pool(name="ps2", bufs=3, space="PSUM") as ps2,
            tc.tile_pool(name="warm", bufs=1) as warmp,
            tc.tile_pool(name="warmps", bufs=1, space="PSUM") as warmpsp,
        ):
            # PE warm-up: ~5us of dummy matmuls while the first DMAs land,
            # so the HAM clock gate is at 8/8 when real matmuls start.
            wtile = warmp.tile([P, TB], mybir.dt.bfloat16)
            nc.vector.memset(wtile[:], 0.0)
            wps = warmpsp.tile([P, TB], f32)
            for _ in range(10):
                nc.tensor.matmul(wps[:], lhsT=wtile[:, :P], rhs=wtile[:],
                                 start=True, stop=True)
            for (c0, tb) in blocks:
                x_sb = xp.tile([P, KF, TB], dt_in, tag="x")
                # One DMA per k-subtile: spreads across queues and lets the
                # first matmuls start as soon as subtile 0 lands.
                for kx in range(KF):
                    nc.sync.dma_start(x_sb[:, kx, :tb], xT[:, kx, c0:c0 + tb])
                act_sb = actp.tile([P, KH, TB], dt_in, tag="act")
                for i in range(KH):
                    wgt_g = wgp.tile([P, KF, P], dt_in, tag="wg")
                    wgt_l = wgp.tile([P, KF, P], dt_in, tag="wg")
                    nc.sync.dma_start(wgt_g[:], wg[:, 2 * i])
                    nc.sync.dma_start(wgt_l[:], wg[:, 2 * i + 1])
                    pg = ps1.tile([P, TB], f32, tag="ps1")
                    pl = ps1.tile([P, TB], f32, tag="ps1")
                    for k in range(KF):
                        nc.tensor.matmul(
                            pg[:, :tb],
                            lhsT=mm(wgt_g[:, k, :]),
                            rhs=mm(x_sb[:, k, :tb]),
                            start=(k == 0), stop=(k == KF - 1),
                        )
                    for k in range(KF):
                        nc.tensor.matmul(
                            pl[:, :tb],
                            lhsT=mm(wgt_l[:, k, :]),
                            rhs=mm(x_sb[:, k, :tb]),
                            start=(k == 0), stop=(k == KF - 1),
                        )
                    gtmp = gp.tile([P, TB], f32, tag="g")
                    nc.scalar.activation(
                        gtmp[:, :tb], pg[:, :tb],
                        mybir.ActivationFunctionType.Gelu_apprx_tanh,
                    )
                    nc.vector.tensor_mul(
                        out=act_sb[:, i, :tb], in0=gtmp[:, :tb], in1=pl[:, :tb]
                    )
                for m in range(MO):
                    wlt = wlp.tile([P, KH, P], dt_in, tag="wl")
                    nc.sync.dma_start(wlt[:], wl[:, m])
                    p2 = ps2.tile([P, TB], f32, tag="ps2")
                    for k in range(KH):
                        nc.tensor.matmul(
                            p2[:, :tb],
                            lhsT=mm(wlt[:, k, :]),
                            rhs=mm(act_sb[:, k, :tb]),
                            start=(k == 0), stop=(k == KH - 1),
                        )
                    o_sb = op.tile([P, TB], f32, tag="o")
                    nc.vector.tensor_copy(out=o_sb[:, :tb], in_=p2[:, :tb])
                    nc.sync.dma_start(yT[m][:, c0:c0 + tb], o_sb[:, :tb])
    nc.compile()
    return nc


def _even_chunks(C, limit):
    """Split [0, C) into near-equal even-sized chunks of size <= limit."""
    assert C % 2 == 0
    nblk = -(-C // limit)
    half_base, half_extra = divmod(C // 2, nblk)
    out = []
    c0 = 0
    for b in range(nblk):
        tb = 2 * (half_base + (1 if b < half_extra else 0))
        out.append((c0, tb))
        c0 += tb
    assert c0 == C
    return out


def _build_nc_dram_act(C, F, H, dtype_name):
    """Variant that stages the GLU activations through DRAM.

    Phase 1 loops wg column-tiles outermost over ALL tokens (wg streamed
    exactly once, x SBUF-resident), writing act to a DRAM scratch tensor.
    Phase 2 loops token chunks, re-streaming only the smaller wl.

    Trace-driven tuning vs the first version of this structure:
    * x loaded with 4 k-subtiles per DMA (20 instructions instead of 80) —
      HWDGE issue costs ~0.6us per dma_start regardless of size, and the
      80-instruction version starved the PE for ~33us at startup.
    * act quarters 0/1 and the first 8 wl tiles are prefetched during
      phase 1 (their pools are allocated at top level so no pool-release
      dependency delays them) — removes the phase-boundary PE gap and the
      HAM re-throttle that followed it.
    * PSUM pools are scoped per phase: ps1 (7 banks) in phase 1, ps2
      (8 banks) in phase 2 — the deeper rings absorb consumer jitter
      that previously stalled matmul groups.
    * Output partials are written as ONE fp16 DMA per (quarter, m) from a
      whole-token SBUF staging tile — 64 DMAs x 0.54MB instead of 320 x
      0.22MB fp32 — and summed in fp32 on host.
    """
    import concourse.mybir as mybir
    import concourse.tile as tile
    from concourse import bacc

    KF = F // P
    KH = H // P
    MG = 2 * H // P
    MO = F // P
    f32 = mybir.dt.float32
    dt_in = _mm_dt(mybir, dtype_name)

    NQ = 4               # phase-2 contraction split: H quarters
    KQ = KH // NQ
    nc = bacc.Bacc(None, target_bir_lowering=False)
    xT = nc.dram_tensor("xT", [P, KF, C], dt_in, kind="ExternalInput")
    wg = nc.dram_tensor("wg", [P, MG, KF, P], dt_in, kind="ExternalInput")
    wl = nc.dram_tensor("wl", [P, MO, KH, P], dt_in, kind="ExternalInput")
    # NQ partial outputs (one per H-quarter), summed in fp32 on host.
    yT = nc.dram_tensor("yT", [NQ, MO, P, C], dt_in, kind="ExternalOutput")

    chunks = _even_chunks(C, TB)

    with tile.TileContext(nc) as tc:
        with (
            tc.tile_pool(name="dram", bufs=1, space="DRAM") as dramp,
            tc.tile_pool(name="acp", bufs=2) as acp,
            tc.tile_pool(name="wlp", bufs=8) as wlp,
            tc.tile_pool(name="op", bufs=3) as op,
        ):
            actd = dramp.tile([KH, P, C], dt_in)

            acp_tiles = {}

            def emit_act_load(q):
                # Whole-token act quarter; scalar HWDGE ring keeps these
                # off the wl/output (sync) ring.
                t = acp.tile([P, KQ, C], dt_in, tag="a")
                for ki in range(KQ):
                    nc.scalar.dma_start(t[:, ki, :], actd[q * KQ + ki][:, :])
                acp_tiles[q] = t

            wl_pre = {}

            def emit_wl_load(q, m):
                t = wlp.tile([P, KQ, P], dt_in, tag="wl")
                nc.sync.dma_start(t[:], wl[:, m, q * KQ:(q + 1) * KQ])
                wl_pre[(q, m)] = t

            with (
                tc.tile_pool(name="xp", bufs=1) as xp,
                tc.tile_pool(name="wgp", bufs=4) as wgp,
                tc.tile_pool(name="gp", bufs=3) as gp,
                tc.tile_pool(name="oa", bufs=4) as oap,
                tc.tile_pool(name="ps1", bufs=7, space="PSUM") as ps1,
                tc.tile_pool(name="warm", bufs=1) as warmp,
                tc.tile_pool(name="warmps", bufs=1, space="PSUM") as warmpsp,
            ):
                # PE warm-up: dummy matmuls while the first DMAs land, so
                # the HAM clock gate is at 8/8 when real matmuls start.
                wtile = warmp.tile([P, TB], mybir.dt.bfloat16)
                nc.vector.memset(wtile[:], 0.0)
                wps = warmpsp.tile([P, TB], f32)
                for _ in range(20):
                    nc.tensor.matmul(wps[:], lhsT=wtile[:, :P], rhs=wtile[:],
                                     start=True, stop=True)
                x_sb = xp.tile([P, KF, C], dt_in)
                # Chunk-major loads so the first token chunk is ready early.
                # Only chunks 0-1 are issued upfront; later chunks are
                # emitted between units so the ACT queue reaches the first
                # gelus quickly (each dma_start issue occupies ACT ~0.6us).
                XK = 4

                def emit_x_load(c, xk=XK):
                    c0, tb = chunks[c]
                    for j in range(0, KF, xk):
                        nc.scalar.dma_start(x_sb[:, j:j + xk, c0:c0 + tb],
                                            xT[:, j:j + xk, c0:c0 + tb])

                # Pair-0 weights are loaded here, interleaved with the x
                # chunks so both HWDGE rings deliver the startup-critical
                # bytes in consumption order: sync carries the gate tile
                # (finely sliced, needed first) while ACT carries chunk-0 x,
                # then the lin tile, then chunk-1 x. Without this, chunk-1
                # x competed for HBM ahead of the lin tile (needed sooner).
                wgt_g0 = wgp.tile([P, KF, P], dt_in, tag="wg")
                wgt_l0 = wgp.tile([P, KF, P], dt_in, tag="wg")
                for j in range(0, KF, 4):
                    nc.sync.dma_start(wgt_g0[:, j:j + 4, :],
                                      wg[:, 0, j:j + 4, :])
                emit_x_load(0)
                nc.scalar.dma_start(wgt_l0[:], wg[:, 1])
                if len(chunks) > 1:
                    emit_x_load(1)
                # Pairs 0 and 1 interleave across chunks: the PE then consumes
                # x chunks at half rate, staying behind the inbound x DMA
                # instead of stalling on it.
                # NI=2: a third interleaved pair was measured slightly WORSE
                # (its extra 1MB weight prefix competes with x for HBM during
                # the bandwidth-critical first ~20us).
                NI = min(2, KH)
                units = [(i, c) for c in range(len(chunks)) for i in range(NI)]
                units += [(i, c) for i in range(NI, KH)
                          for c in range(len(chunks))]
                pair_tiles = {0: (wgt_g0, wgt_l0)}
                done = {}
                for (i, c) in units:
                    if i not in pair_tiles:
                        wgt_g = wgp.tile([P, KF, P], dt_in, tag="wg")
                        wgt_l = wgp.tile([P, KF, P], dt_in, tag="wg")
                        nc.sync.dma_start(wgt_g[:], wg[:, 2 * i])
                        nc.sync.dma_start(wgt_l[:], wg[:, 2 * i + 1])
                        pair_tiles[i] = (wgt_g, wgt_l)
                    wgt_g, wgt_l = pair_tiles[i]
                    for (c0, tb) in [chunks[c]]:
                        pg = ps1.tile([P, TB], f32, tag="ps1")
                        pl = ps1.tile([P, TB], f32, tag="ps1")
                        for k in range(KF):
                            nc.tensor.matmul(
                                pg[:, :tb], lhsT=wgt_g[:, k, :],
                                rhs=x_sb[:, k, c0:c0 + tb],
                                start=(k == 0), stop=(k == KF - 1),
                            )
                        for k in range(KF):
                            nc.tensor.matmul(
                                pl[:, :tb], lhsT=wgt_l[:, k, :],
                                rhs=x_sb[:, k, c0:c0 + tb],
                                start=(k == 0), stop=(k == KF - 1),
                            )
                        gtmp = gp.tile([P, TB], f32, tag="g")
                        nc.scalar.activation(
                            gtmp[:, :tb], pg[:, :tb],
                            mybir.ActivationFunctionType.Gelu_apprx_tanh,
                        )
                        oa = oap.tile([P, TB], dt_in, tag="oa")
                        nc.vector.tensor_mul(
                            out=oa[:, :tb], in0=gtmp[:, :tb], in1=pl[:, :tb]
                        )
                        nc.sync.dma_start(actd[i][:, c0:c0 + tb], oa[:, :tb])
                    if i == NI - 1 and c + 2 < len(chunks):
                        emit_x_load(c + 2)
                    done[i] = done.get(i, 0) + 1
                    # Prefetch phase-2 inputs as soon as their act rows are
                    # fully written: quarter 0 (plus the first wl tiles)
                    # after i=KQ-1, quarter 1 after i=2*KQ-1.
                    if done[i] == len(chunks) and i == KQ - 1 \
                            and all(done.get(i2, 0) == len(chunks)
                                    for i2 in range(KQ)):
                        emit_act_load(0)
                        for m in range(min(8, MO)):
                            emit_wl_load(0, m)
                    elif done[i] == len(chunks) and i == 2 * KQ - 1 \
                            and all(done.get(i2, 0) == len(chunks)
                                    for i2 in range(2 * KQ)):
                        emit_act_load(1)
            with tc.tile_pool(name="ps2", bufs=8, space="PSUM") as ps2:
                for q in range(NQ):
                    a_sb = acp_tiles.pop(q)
                    for m in range(MO):
                        wlt = wl_pre.pop((q, m), None)
                        if wlt is None:
                            wlt = wlp.tile([P, KQ, P], dt_in, tag="wl")
                            nc.sync.dma_start(wlt[:],
                                              wl[:, m, q * KQ:(q + 1) * KQ])
                        o_sb = op.tile([P, C], dt_in, tag="o")
                        last = (q == NQ - 1 and m == MO - 1)
                        for (c0, tb) in chunks:
                            p2 = ps2.tile([P, TB], f32, tag="ps2")
                            for k in range(KQ):
                                nc.tensor.matmul(
                                    p2[:, :tb], lhsT=wlt[:, k, :],
                                    rhs=a_sb[:, k, c0:c0 + tb],
                                    start=(k == 0), stop=(k == KQ - 1),
                                )
                            nc.vector.tensor_copy(out=o_sb[:, c0:c0 + tb],
                                                  in_=p2[:, :tb])
                            if last:
                                # Per-chunk writes for the final tile so the
                                # closing DMA drains during the last matmuls.
                                nc.sync.dma_start(yT[q][m][:, c0:c0 + tb],
                                                  o_sb[:, c0:c0 + tb])
                        if not last:
                            nc.sync.dma_start(yT[q][m][:, :], o_sb[:])
                    # Prefetch quarter q+2 once quarter q's readers are
                    # emitted (its acp buffer is the one being recycled).
                    if q + 2 < NQ:
                        emit_act_load(q + 2)
    nc.compile()
    return nc


MOE_STRUCT = os.environ.get("MOE_STRUCT", "dram_act")


def _get_nc(C, F, H, dtype_name):
    key = (C, F, H, dtype_name, TB, MOE_STRUCT)
    if key not in _NEFF_CACHE:
        build = _build_nc_dram_act if MOE_STRUCT == "dram_act" else _build_nc
        _NEFF_CACHE[key] = build(C, F, H, dtype_name)
    return _NEFF_CACHE[key]


def _mm_dt(mybir, dtype_name):
    return {
        "f32r": mybir.dt.float32r,
        "bf16": mybir.dt.bfloat16,
        "f16": mybir.dt.float16,
    }[dtype_name]


def _np_in_dtype():
    if MOE_DTYPE == "f32r":
        return np.float32
    if MOE_DTYPE == "f16":
        return np.float16
    import ml_dtypes

    return ml_dtypes.bfloat16


def run(x, w_router, w_gating, w_linear, per_expert_scale, router_scale, top_k,
        trace=False):
    from concourse.bass_utils import run_bass_kernel_spmd

    x = np.asarray(x)
    w_router = np.asarray(w_router)
    w_gating = np.asarray(w_gating)
    w_linear = np.asarray(w_linear)
    per_expert_scale = np.asarray(per_expert_scale)
    router_scale = np.asarray(router_scale)
    k = int(top_k)

    G, S, F = x.shape
    E = w_router.shape[1]
    H = w_linear.shape[1]
    B = G * S
    assert E == 8, "expert-parallel mapping assumes 8 experts on 8 cores"
    KF, KH, MO = F // P, H // P, F // P

    choices, combine = _route(x, w_router, router_scale, k)
    wcopy = combine * per_expert_scale.astype(np.float32)[choices]

    cf = choices.reshape(-1)
    tok_of_copy = np.repeat(np.arange(B), k)
    idx_per_e = [np.nonzero(cf == e)[0] for e in range(E)]
    counts = np.array([len(ix) for ix in idx_per_e])
    # Pad only to even (fp16 path): PE work scales with C, so every token
    # of padding costs ~0.64us across the two stages.
    C = max(512, int(-(-counts.max() // 2)) * 2)

    nc = _get_nc(C, F, H, MOE_DTYPE)
    dt_in = _np_in_dtype()

    xf = x.reshape(B, F)
    in_maps = []
    toks_per_e = []
    for e in range(E):
        toks = tok_of_copy[idx_per_e[e]]
        toks_per_e.append(toks)
        n_e = len(toks)
        # xT [P, KF, C]: xT[p, ko, c] = x[toks[c], ko*P + p]
        xT = np.zeros((P, KF, C), dtype=dt_in)
        xT[:, :, :n_e] = (
            xf[toks].astype(dt_in).reshape(n_e, KF, P).transpose(2, 1, 0)
        )
        # wg [P, MG, KF, P]: m=2i+c -> gate (c=0) / lin (c=1) rows 128i..128i+127
        wgq = w_gating[e].reshape(2, KH, P, KF, P)        # (c, i, col, ko, p)
        wgt = np.ascontiguousarray(
            wgq.transpose(4, 1, 0, 3, 2).reshape(P, 2 * KH, KF, P)
        ).astype(dt_in)
        # wl [P, MO, KH, P]: wl[p, m, kh, col] = w_linear[e][kh*P+p, m*P+col]
        wlq = w_linear[e].reshape(KH, P, MO, P)           # (kh, p, m, col)
        wlt = np.ascontiguousarray(wlq.transpose(1, 2, 0, 3)).astype(dt_in)
        in_maps.append({"xT": xT, "wg": wgt, "wl": wlt})

    res = run_bass_kernel_spmd(
        nc, in_maps, core_ids=list(range(E)), trace=trace,
        trace_cores=list(range(E)) if trace else None,
    )

    out = np.zeros((B, F), dtype=np.float32)
    for e in range(E):
        toks = toks_per_e[e]
        n_e = len(toks)
        if n_e == 0:
            continue
        yT = res.results[e]["yT"]                         # [MO, P, C] or [NQ, MO, P, C]
        if yT.ndim == 4:
            yT = yT.sum(axis=0, dtype=np.float32)
        y = yT.transpose(2, 0, 1).reshape(C, F)[:n_e]
        w = wcopy.reshape(-1)[idx_per_e[e]][:, None]
        out[toks] += w * y
    return out.reshape(G, S, F), res


def kernel(**inputs) -> np.ndarray:
    out, _ = run(**inputs)
    return out



# revision 21
# speedup vs baseline: 1.0418x; 1.0418x over previous
"""MoE ragged FFN kernel for Trainium2 (8 NeuronCores, expert-parallel).

Strategy
--------
* Router (RMSNorm -> scaled projection -> softmax -> top-k -> renorm) is
  computed on host with jax-on-CPU using exactly the reference ops, so the
  discrete expert choices match the reference bit-for-bit.
* Expert-parallel sharding: core e owns expert e's weights. Tokens routed to
  expert e are gathered on host, padded to a common capacity C, and shipped
  pre-transposed so the device only runs dense matmuls.
* Device (per core): Y1^T = Wg^T @ X^T (contraction F), GLU
  act = gelu_tanh(gate) * lin computed pairwise on 128-row tiles,
  Y2^T = Wl^T @ act (contraction H). Tokens are always the matmul moving/free
  dimension; features live on partitions. fp16 matmul inputs (value ranges
  here are far from fp16 limits; 8x finer quantization than bf16 at the same
  1 cycle/row PE rate), fp32 PSUM accumulate.
* Default structure ("dram_act") stages activations through DRAM: stage-1
  streams wg exactly once with x SBUF-resident; stage-2 splits H into
  quarters with the act quarter SBUF-resident, wl streamed once, and four
  fp16 partial outputs summed in fp32 on host.
* Trace-driven scheduling: batched x loads (HWDGE issue is ~0.6us per
  dma_start regardless of size), phase-2 act/wl prefetch during phase 1
  (pools hoisted to top level to avoid pool-release dependencies),
  per-phase PSUM pool scoping (7 banks stage 1 / 8 banks stage 2), PE
  warm-up matmuls sized to bridge until the first x chunk lands.
* Host combines: out[token] += combine_weight * per_expert_scale[e] * y.

Measured (8 cores, G=4 S=2048 F=2048 H=4096 E=8 k=2): HW exec ~1.42 ms
(PE idle <5us, ~96.4% MFU, ~104% of the C=2142 single-core matmul-stream
floor), global L2 relative error ~5.5e-4 vs the fp32 reference.
"""

import os

import numpy as np

P = 128
RMS_EPS = 1e-6

# Matmul input precision: "f16" (default), "bf16", or "f32r".
MOE_DTYPE = os.environ.get("MOE_DTYPE", "f16")
# Token-block size (matmul moving free dim; >=256 keeps fp32r at 1 cyc/row).
TB = int(os.environ.get("MOE_TB", "512"))

_NEFF_CACHE: dict = {}


def _route_numpy(x, w_router, router_scale, top_k):
    """Fallback router in numpy (used only if jax-on-CPU is unavailable)."""
    G, S, F = x.shape
    B = G * S
    var = np.mean(np.square(x), axis=-1, keepdims=True, dtype=np.float32)
    ri = x / np.sqrt(var + RMS_EPS)
    ri = ri * np.float32(1.0 / np.sqrt(np.float32(F))) * router_scale
    logits = (ri.reshape(B, F) @ w_router).astype(np.float32)
    m = logits.max(axis=-1, keepdims=True)
    e = np.exp(logits - m)
    probs = e / e.sum(axis=-1, keepdims=True)
    choices = np.argsort(-logits, axis=-1, kind="stable")[:, :top_k]
    sel = np.take_along_axis(probs, choices, axis=-1)
    renorm = sel.sum(axis=-1, keepdims=True)
    renorm = np.where(renorm > 0.0, renorm, np.float32(1.0))
    combine = (sel / renorm).astype(np.float32)
    return choices.astype(np.int64), combine


def _route(x, w_router, router_scale, top_k):
    """Reference-exact router on CPU via jax. Returns (choices, combine) as
    numpy arrays of shape (B, k)."""
    try:
        import jax
        import jax.numpy as jnp

        cpu = jax.devices("cpu")[0]
    except Exception:
        return _route_numpy(np.asarray(x, dtype=np.float32),
                            np.asarray(w_router), np.asarray(router_scale),
                            top_k)
    G, S, F = x.shape
    E = w_router.shape[1]
    with jax.default_device(cpu):
        xj = jax.device_put(np.asarray(x), cpu)
        wj = jax.device_put(np.asarray(w_router), cpu)
        rj = jax.device_put(np.asarray(router_scale), cpu)
        var = jnp.mean(jnp.square(xj), axis=-1, keepdims=True)
        ri = xj * jax.lax.rsqrt(var + RMS_EPS)
        root_size = jax.lax.rsqrt(jnp.array(F, dtype=ri.dtype))
        ri = ri * root_size * rj.astype(ri.dtype)
        logits = jnp.einsum("gsd,de->gse", ri, wj).astype(jnp.float32)
        probs = jax.nn.softmax(logits, axis=-1)
        _, choices = jax.lax.approx_max_k(logits, k=top_k)
        indicator = jax.nn.one_hot(choices, E, dtype=probs.dtype).sum(axis=-2)
        renorm = jnp.sum(indicator * probs, axis=-1, keepdims=True)
        renorm = jnp.where(renorm > 0.0, renorm, 1.0)
        weights = probs / renorm
        combine = jnp.take_along_axis(weights, choices, axis=-1)
    B = G * S
    return (
        np.asarray(choices).reshape(B, top_k),
        np.asarray(combine).reshape(B, top_k).astype(np.float32),
    )


def _build_nc(C, F, H, dtype_name):
    """Build + compile the per-core FFN program (same program on all cores)."""
    import concourse.mybir as mybir
    import concourse.tile as tile
    from concourse import bacc

    KF = F // P          # k-subtiles for stage 1 (contraction F)
    KH = H // P          # k-subtiles for stage 2 (contraction H)
    MG = 2 * H // P      # wg column tiles, gate/lin interleaved per 128
    MO = F // P          # output row tiles
    f32 = mybir.dt.float32
    dt_in = _mm_dt(mybir, dtype_name)
    dt_mm = dt_in

    def mm(ap):
        return ap.bitcast(dt_mm) if ap.dtype != dt_mm else ap

    nc = bacc.Bacc(None, target_bir_lowering=False)
    xT = nc.dram_tensor("xT", [P, KF, C], dt_in, kind="ExternalInput")
    wg = nc.dram_tensor("wg", [P, MG, KF, P], dt_in, kind="ExternalInput")
    wl = nc.dram_tensor("wl", [P, MO, KH, P], dt_in, kind="ExternalInput")
    yT = nc.dram_tensor("yT", [MO, P, C], f32, kind="ExternalOutput")

    # Equal-size blocks (<= TB): avoids a tiny LDWEIGHTS-bound tail block.
    # Sizes are kept even (fp32r matmuls require an even moving free dim).
    assert C % 2 == 0
    nblk = -(-C // TB)
    half_base, half_extra = divmod(C // 2, nblk)
    blocks = []
    c0 = 0
    for b in range(nblk):
        tb = 2 * (half_base + (1 if b < half_extra else 0))
        blocks.append((c0, tb))
        c0 += tb
    assert c0 == C

    # f32r tiles are 2x the size of bf16 — shrink pools to fit SBUF.
    wbufs = 4 if dtype_name != "f32r" else 2
    abufs = 2 if dtype_name != "f32r" else 1
    with tile.TileContext(nc) as tc:
        with (
            tc.tile_pool(name="xp", bufs=2) as xp,
            tc.tile_pool(name="wgp", bufs=wbufs) as wgp,
            tc.tile_pool(name="wlp", bufs=wbufs) as wlp,
            tc.tile_pool(name="actp", bufs=abufs) as actp,
            tc.tile_pool(name="gp", bufs=3) as gp,
            tc.tile_pool(name="op", bufs=3) as op,
            tc.tile_pool(name="ps1", bufs=4, space="PSUM") as ps1,
            tc.tile_pool(name="ps2", bufs=3, space="PSUM") as ps2,
            tc.tile_pool(name="warm", bufs=1) as warmp,
            tc.tile_pool(name="warmps", bufs=1, space="PSUM") as warmpsp,
        ):
            # PE warm-up: ~5us of dummy matmuls while the first DMAs land,
            # so the HAM clock gate is at 8/8 when real matmuls start.
            wtile = warmp.tile([P, TB], mybir.dt.bfloat16)
            nc.vector.memset(wtile[:], 0.0)
            wps = warmpsp.tile([P, TB], f32)
            for _ in range(10):
                nc.tensor.matmul(wps[:], lhsT=wtile[:, :P], rhs=wtile[:],
                                 start=True, stop=True)
            for (c0, tb) in blocks:
                x_sb = xp.tile([P, KF, TB], dt_in, tag="x")
                # One DMA per k-subtile: spreads across queues and lets the
                # first matmuls start as soon as subtile 0 lands.
                for kx in range(KF):
                    nc.sync.dma_start(x_sb[:, kx, :tb], xT[:, kx, c0:c0 + tb])
                act_sb = actp.tile([P, KH, TB], dt_in, tag="act")
                for i in range(KH):
                    wgt_g = wgp.tile([P, KF, P], dt_in, tag="wg")
                    wgt_l = wgp.tile([P, KF, P], dt_in, tag="wg")
                    nc.sync.dma_start(wgt_g[:], wg[:, 2 * i])
                    nc.sync.dma_start(wgt_l[:], wg[:, 2 * i + 1])
                    pg = ps1.tile([P, TB], f32, tag="ps1")
                    pl = ps1.tile([P, TB], f32, tag="ps1")
                    for k in range(KF):
                        nc.tensor.matmul(
                            pg[:, :tb],
                            lhsT=mm(wgt_g[:, k, :]),
                            rhs=mm(x_sb[:, k, :tb]),
                            start=(k == 0), stop=(k == KF - 1),
                        )
                    for k in range(KF):
                        nc.tensor.matmul(
                            pl[:, :tb],
                            lhsT=mm(wgt_l[:, k, :]),
                            rhs=mm(x_sb[:, k, :tb]),
                            start=(k == 0), stop=(k == KF - 1),
                        )
                    gtmp = gp.tile([P, TB], f32, tag="g")
                    nc.scalar.activation(
                        gtmp[:, :tb], pg[:, :tb],
                        mybir.ActivationFunctionType.Gelu_apprx_tanh,
                    )
                    nc.vector.tensor_mul(
                        out=act_sb[:, i, :tb], in0=gtmp[:, :tb], in1=pl[:, :tb]
                    )
                for m in range(MO):
                    wlt = wlp.tile([P, KH, P], dt_in, tag="wl")
                    nc.sync.dma_start(wlt[:], wl[:, m])
                    p2 = ps2.tile([P, TB], f32, tag="ps2")
                    for k in range(KH):
                        nc.tensor.matmul(
                            p2[:, :tb],
                            lhsT=mm(wlt[:, k, :]),
                            rhs=mm(act_sb[:, k, :tb]),
                            start=(k == 0), stop=(k == KH - 1),
                        )
                    o_sb = op.tile([P, TB], f32, tag="o")
                    nc.vector.tensor_copy(out=o_sb[:, :tb], in_=p2[:, :tb])
                    nc.sync.dma_start(yT[m][:, c0:c0 + tb], o_sb[:, :tb])
    nc.compile()
    return nc


def _even_chunks(C, limit):
    """Split [0, C) into near-equal even-sized chunks of size <= limit."""
    assert C % 2 == 0
    nblk = -(-C // limit)
    half_base, half_extra = divmod(C // 2, nblk)
    out = []
    c0 = 0
    for b in range(nblk):
        tb = 2 * (half_base + (1 if b < half_extra else 0))
        out.append((c0, tb))
        c0 += tb
    assert c0 == C
    return out


def _build_nc_dram_act(C, F, H, dtype_name):
    """Variant that stages the GLU activations through DRAM.

    Phase 1 loops wg column-tiles outermost over ALL tokens (wg streamed
    exactly once, x SBUF-resident), writing act to a DRAM scratch tensor.
    Phase 2 loops token chunks, re-streaming only the smaller wl.

    Trace-driven tuning vs the first version of this structure:
    * x loaded with 4 k-subtiles per DMA (20 instructions instead of 80) —
      HWDGE issue costs ~0.6us per dma_start regardless of size, and the
      80-instruction version starved the PE for ~33us at startup.
    * act quarters 0/1 and the first 8 wl tiles are prefetched during
      phase 1 (their pools are allocated at top level so no pool-release
      dependency delays them) — removes the phase-boundary PE gap and the
      HAM re-throttle that followed it.
    * PSUM pools are scoped per phase: ps1 (7 banks) in phase 1, ps2
      (8 banks) in phase 2 — the deeper rings absorb consumer jitter
      that previously stalled matmul groups.
    * Output partials are written as ONE fp16 DMA per (quarter, m) from a
      whole-token SBUF staging tile — 64 DMAs x 0.54MB instead of 320 x
      0.22MB fp32 — and summed in fp32 on host.
    """
    import concourse.mybir as mybir
    import concourse.tile as tile
    from concourse import bacc

    KF = F // P
    KH = H // P
    MG = 2 * H // P
    MO = F // P
    f32 = mybir.dt.float32
    dt_in = _mm_dt(mybir, dtype_name)

    NQ = 4               # phase-2 contraction split: H quarters
    KQ = KH // NQ
    nc = bacc.Bacc(None, target_bir_lowering=False)
    xT = nc.dram_tensor("xT", [P, KF, C], dt_in, kind="ExternalInput")
    wg = nc.dram_tensor("wg", [P, MG, KF, P], dt_in, kind="ExternalInput")
    wl = nc.dram_tensor("wl", [P, MO, KH, P], dt_in, kind="ExternalInput")
    # NQ partial outputs (one per H-quarter), summed in fp32 on host.
    yT = nc.dram_tensor("yT", [NQ, MO, P, C], dt_in, kind="ExternalOutput")

    chunks = _even_chunks(C, TB)

    with tile.TileContext(nc) as tc:
        with (
            tc.tile_pool(name="dram", bufs=1, space="DRAM") as dramp,
            tc.tile_pool(name="acp", bufs=2) as acp,
            tc.tile_pool(name="wlp", bufs=8) as wlp,
            tc.tile_pool(name="op", bufs=3) as op,
        ):
            actd = dramp.tile([KH, P, C], dt_in)

            acp_tiles = {}

            def emit_act_load(q):
                # Whole-token act quarter; scalar HWDGE ring keeps these
                # off the wl/output (sync) ring.
                t = acp.tile([P, KQ, C], dt_in, tag="a")
                for ki in range(KQ):
                    nc.scalar.dma_start(t[:, ki, :], actd[q * KQ + ki][:, :])
                acp_tiles[q] = t

            wl_pre = {}

            def emit_wl_load(q, m):
                t = wlp.tile([P, KQ, P], dt_in, tag="wl")
                nc.sync.dma_start(t[:], wl[:, m, q * KQ:(q + 1) * KQ])
                wl_pre[(q, m)] = t

            with (
                tc.tile_pool(name="xp", bufs=1) as xp,
                tc.tile_pool(name="wgp", bufs=4) as wgp,
                tc.tile_pool(name="gp", bufs=3) as gp,
                tc.tile_pool(name="oa", bufs=4) as oap,
                tc.tile_pool(name="ps1", bufs=7, space="PSUM") as ps1,
                tc.tile_pool(name="warm", bufs=1) as warmp,
                tc.tile_pool(name="warmps", bufs=1, space="PSUM") as warmpsp,
            ):
                # PE warm-up: dummy matmuls while the first DMAs land, so
                # the HAM clock gate is at 8/8 when real matmuls start.
                wtile = warmp.tile([P, TB], mybir.dt.bfloat16)
                nc.vector.memset(wtile[:], 0.0)
                wps = warmpsp.tile([P, TB], f32)
                # 12 x 512-row bf16 warm matmuls ~= one 3.4us HAM window at
                # the cold clock: enough to reach K=8/8 and bridge until the
                # first x/wg bytes land (~11us with the ring-ordered loads);
                # more than that delays the first real matmul.
                for _ in range(12):
                    nc.tensor.matmul(wps[:], lhsT=wtile[:, :P], rhs=wtile[:],
                                     start=True, stop=True)
                x_sb = xp.tile([P, KF, C], dt_in)
                # Chunk-major loads so the first token chunk is ready early.
                # Only chunks 0-1 are issued upfront; later chunks are
                # emitted between units so the ACT queue reaches the first
                # gelus quickly (each dma_start issue occupies ACT ~0.6us).
                XK = 4

                def emit_x_load(c, xk=XK):
                    c0, tb = chunks[c]
                    for j in range(0, KF, xk):
                        nc.scalar.dma_start(x_sb[:, j:j + xk, c0:c0 + tb],
                                            xT[:, j:j + xk, c0:c0 + tb])

                # Pair-0 weights are loaded here, interleaved with the x
                # chunks so both HWDGE rings deliver the startup-critical
                # bytes in consumption order: sync carries the gate tile
                # (finely sliced, needed first) while ACT carries chunk-0 x,
                # then the lin tile, then chunk-1 x. Without this, chunk-1
                # x competed for HBM ahead of the lin tile (needed sooner).
                wgt_g0 = wgp.tile([P, KF, P], dt_in, tag="wg")
                wgt_l0 = wgp.tile([P, KF, P], dt_in, tag="wg")
                for j in range(0, KF, 4):
                    nc.sync.dma_start(wgt_g0[:, j:j + 4, :],
                                      wg[:, 0, j:j + 4, :])
                emit_x_load(0)
                nc.scalar.dma_start(wgt_l0[:], wg[:, 1])
                if len(chunks) > 1:
                    emit_x_load(1)
                # Pairs 0 and 1 interleave across chunks: the PE then consumes
                # x chunks at half rate, staying behind the inbound x DMA
                # instead of stalling on it.
                # NI=2: a third interleaved pair was measured slightly WORSE
                # (its extra 1MB weight prefix competes with x for HBM during
                # the bandwidth-critical first ~20us).
                NI = min(2, KH)
                units = [(i, c) for c in range(len(chunks)) for i in range(NI)]
                units += [(i, c) for i in range(NI, KH)
                          for c in range(len(chunks))]
                pair_tiles = {0: (wgt_g0, wgt_l0)}
                done = {}
                for (i, c) in units:
                    if i not in pair_tiles:
                        wgt_g = wgp.tile([P, KF, P], dt_in, tag="wg")
                        wgt_l = wgp.tile([P, KF, P], dt_in, tag="wg")
                        nc.sync.dma_start(wgt_g[:], wg[:, 2 * i])
                        nc.sync.dma_start(wgt_l[:], wg[:, 2 * i + 1])
                        pair_tiles[i] = (wgt_g, wgt_l)
                    wgt_g, wgt_l = pair_tiles[i]
                    for (c0, tb) in [chunks[c]]:
                        pg = ps1.tile([P, TB], f32, tag="ps1")
                        pl = ps1.tile([P, TB], f32, tag="ps1")
                        for k in range(KF):
                            nc.tensor.matmul(
                                pg[:, :tb], lhsT=wgt_g[:, k, :],
                                rhs=x_sb[:, k, c0:c0 + tb],
                                start=(k == 0), stop=(k == KF - 1),
                            )
                        for k in range(KF):
                            nc.tensor.matmul(
                                pl[:, :tb], lhsT=wgt_l[:, k, :],
                                rhs=x_sb[:, k, c0:c0 + tb],
                                start=(k == 0), stop=(k == KF - 1),
                            )
                        gtmp = gp.tile([P, TB], f32, tag="g")
                        nc.scalar.activation(
                            gtmp[:, :tb], pg[:, :tb],
                            mybir.ActivationFunctionType.Gelu_apprx_tanh,
                        )
                        oa = oap.tile([P, TB], dt_in, tag="oa")
                        nc.vector.tensor_mul(
                            out=oa[:, :tb], in0=gtmp[:, :tb], in1=pl[:, :tb]
                        )
                        nc.sync.dma_start(actd[i][:, c0:c0 + tb], oa[:, :tb])
                    if i == NI - 1 and c + 2 < len(chunks):
                        emit_x_load(c + 2)
                    done[i] = done.get(i, 0) + 1
                    # Prefetch phase-2 inputs as soon as their act rows are
                    # fully written: quarter 0 (plus the first wl tiles)
                    # after i=KQ-1, quarter 1 after i=2*KQ-1.
                    if done[i] == len(chunks) and i == KQ - 1 \
                            and all(done.get(i2, 0) == len(chunks)
                                    for i2 in range(KQ)):
                        emit_act_load(0)
                        for m in range(min(8, MO)):
                            emit_wl_load(0, m)
                    elif done[i] == len(chunks) and i == 2 * KQ - 1 \
                            and all(done.get(i2, 0) == len(chunks)
                                    for i2 in range(2 * KQ)):
                        emit_act_load(1)
            with tc.tile_pool(name="ps2", bufs=8, space="PSUM") as ps2:
                for q in range(NQ):
                    a_sb = acp_tiles.pop(q)
                    for m in range(MO):
                        wlt = wl_pre.pop((q, m), None)
                        if wlt is None:
                            wlt = wlp.tile([P, KQ, P], dt_in, tag="wl")
                            nc.sync.dma_start(wlt[:],
                                              wl[:, m, q * KQ:(q + 1) * KQ])
                        o_sb = op.tile([P, C], dt_in, tag="o")
                        last = (q == NQ - 1 and m == MO - 1)
                        for (c0, tb) in chunks:
                            p2 = ps2.tile([P, TB], f32, tag="ps2")
                            for k in range(KQ):
                                nc.tensor.matmul(
                                    p2[:, :tb], lhsT=wlt[:, k, :],
                                    rhs=a_sb[:, k, c0:c0 + tb],
                                    start=(k == 0), stop=(k == KQ - 1),
                                )
                            nc.vector.tensor_copy(out=o_sb[:, c0:c0 + tb],
                                                  in_=p2[:, :tb])
                            if last:
                                # Per-chunk writes for the final tile so the
                                # closing DMA drains during the last matmuls.
                                nc.sync.dma_start(yT[q][m][:, c0:c0 + tb],
                                                  o_sb[:, c0:c0 + tb])
                        if not last:
                            nc.sync.dma_start(yT[q][m][:, :], o_sb[:])
                    # Prefetch quarter q+2 once quarter q's readers are
                    # emitted (its acp buffer is the one being recycled).
                    if q + 2 < NQ:
                        emit_act_load(q + 2)
    nc.compile()
    return nc


MOE_STRUCT = os.environ.get("MOE_STRUCT", "dram_act")


def _get_nc(C, F, H, dtype_name):
    key = (C, F, H, dtype_name, TB, MOE_STRUCT)
    if key not in _NEFF_CACHE:
        build = _build_nc_dram_act if MOE_STRUCT == "dram_act" else _build_nc
        _NEFF_CACHE[key] = build(C, F, H, dtype_name)
    return _NEFF_CACHE[key]


def _mm_dt(mybir, dtype_name):
    return {
        "f32r": mybir.dt.float32r,
        "bf16": mybir.dt.bfloat16,
        "f16": mybir.dt.float16,
    }[dtype_name]


def _np_in_dtype():
    if MOE_DTYPE == "f32r":
        return np.float32
    if MOE_DTYPE == "f16":
        return np.float16
    import ml_dtypes

    return ml_dtypes.bfloat16


def run(x, w_router, w_gating, w_linear, per_expert_scale, router_scale, top_k,
        trace=False):
    from concourse.bass_utils import run_bass_kernel_spmd

    x = np.asarray(x)
    w_router = np.asarray(w_router)
    w_gating = np.asarray(w_gating)
    w_linear = np.asarray(w_linear)
    per_expert_scale = np.asarray(per_expert_scale)
    router_scale = np.asarray(router_scale)
    k = int(top_k)

    G, S, F = x.shape
    E = w_router.shape[1]
    H = w_linear.shape[1]
    B = G * S
    assert E == 8, "expert-parallel mapping assumes 8 experts on 8 cores"
    KF, KH, MO = F // P, H // P, F // P

    choices, combine = _route(x, w_router, router_scale, k)
    wcopy = combine * per_expert_scale.astype(np.float32)[choices]

    cf = choices.reshape(-1)
    tok_of_copy = np.repeat(np.arange(B), k)
    idx_per_e = [np.nonzero(cf == e)[0] for e in range(E)]
    counts = np.array([len(ix) for ix in idx_per_e])
    # Pad only to even (fp16 path): PE work scales with C, so every token
    # of padding costs ~0.64us across the two stages.
    C = max(512, int(-(-counts.max() // 2)) * 2)

    nc = _get_nc(C, F, H, MOE_DTYPE)
    dt_in = _np_in_dtype()

    xf = x.reshape(B, F)
    in_maps = []
    toks_per_e = []
    for e in range(E):
        toks = tok_of_copy[idx_per_e[e]]
        toks_per_e.append(toks)
        n_e = len(toks)
        # xT [P, KF, C]: xT[p, ko, c] = x[toks[c], ko*P + p]
        xT = np.zeros((P, KF, C), dtype=dt_in)
        xT[:, :, :n_e] = (
            xf[toks].astype(dt_in).reshape(n_e, KF, P).transpose(2, 1, 0)
        )
        # wg [P, MG, KF, P]: m=2i+c -> gate (c=0) / lin (c=1) rows 128i..128i+127
        wgq = w_gating[e].reshape(2, KH, P, KF, P)        # (c, i, col, ko, p)
        wgt = np.ascontiguousarray(
            wgq.transpose(4, 1, 0, 3, 2).reshape(P, 2 * KH, KF, P)
        ).astype(dt_in)
        # wl [P, MO, KH, P]: wl[p, m, kh, col] = w_linear[e][kh*P+p, m*P+col]
        wlq = w_linear[e].reshape(KH, P, MO, P)           # (kh, p, m, col)
        wlt = np.ascontiguousarray(wlq.transpose(1, 2, 0, 3)).astype(dt_in)
        in_maps.append({"xT": xT, "wg": wgt, "wl": wlt})

    res = run_bass_kernel_spmd(
        nc, in_maps, core_ids=list(range(E)), trace=trace,
        trace_cores=list(range(E)) if trace else None,
    )

    out = np.zeros((B, F), dtype=np.float32)
    for e in range(E):
        toks = toks_per_e[e]
        n_e = len(toks)
        if n_e == 0:
            continue
        yT = res.results[e]["yT"]                         # [MO, P, C] or [NQ, MO, P, C]
        if yT.ndim == 4:
            yT = yT.sum(axis=0, dtype=np.float32)
        y = yT.transpose(2, 0, 1).reshape(C, F)[:n_e]
        w = wcopy.reshape(-1)[idx_per_e[e]][:, None]
        out[toks] += w * y
    return out.reshape(G, S, F), res


def kernel(**inputs) -> np.ndarray:
    out, _ = run(**inputs)
    return out

